# revision 16
# baseline (speedup 1.0000x reference)
"""Trainium2 Bass kernel for nn_AbsSeq2SeqLSTM (bi-LSTM encoder + greedy-argmax LSTM decoder).

Sharding: pure data parallelism — batch 1024 split as 128 per NeuronCore across 8 cores;
all weights replicated.

Per-core device program (all shapes hardcoded for B_local=128, S=128, H=512, T=48):
  - z[b, gates] = hT-stationary x Wh-moving matmuls in float32r (full-rate at N=512),
    input projection folded on host into per-class tables ([16,2048]/[9,2048]) applied
    via one-hot matmuls accumulated into the same PSUM banks.
  - gates on ScalarE (sigmoid/tanh LUT), cell update on VectorE, per-step h transpose
    on TensorE (needed because the next step's stationary operand is h^T).
  - decoder argmax via VectorE max/max_index (first-index ties, matches np.argmax);
    softmax deferred to one batched pass at the end (single exp table load).
"""

import os
import sys

for _p in ("/opt/trn_rl_repo", "/root/.axon_site/_ro/trn_rl_repo"):
    if os.path.isdir(_p) and _p not in sys.path:
        sys.path.append(_p)

import numpy as np
import concourse.bass as bass
import concourse.tile as tile
from concourse import bacc, mybir
from concourse.bass_utils import run_bass_kernel_spmd

N_CORES = 8
B_LOC = 128
H = 512
G = 2048
VIN = 16
VOUT = 9

f32 = mybir.dt.float32
f32r = mybir.dt.float32r
bf16 = mybir.dt.bfloat16
RECUR_BF16 = os.environ.get("K_RECUR_BF16", "0") == "1"
rdt = bf16 if RECUR_BF16 else mybir.dt.float32r
i32 = mybir.dt.int32
u32 = mybir.dt.uint32
AF = mybir.ActivationFunctionType
ALU = mybir.AluOpType
AX = mybir.AxisListType


def build_program(S=128, T=48):
    nc = bacc.Bacc("TRN2", target_bir_lowering=False, debug=False)

    TOK = nc.dram_tensor("tokens", [B_LOC, S], i32, kind="ExternalInput").ap()
    WH = {
        "f": nc.dram_tensor("whf", [H, G], f32, kind="ExternalInput").ap(),
        "b": nc.dram_tensor("whb", [H, G], f32, kind="ExternalInput").ap(),
        "d": nc.dram_tensor("whd", [H, G], f32, kind="ExternalInput").ap(),
    }
    ZX = {
        "f": nc.dram_tensor("zxf", [VIN, G], f32, kind="ExternalInput").ap(),
        "b": nc.dram_tensor("zxb", [VIN, G], f32, kind="ExternalInput").ap(),
    }
    ZED = nc.dram_tensor("zed", [VOUT, G], f32, kind="ExternalInput").ap()
    WOUT = nc.dram_tensor("wout", [H, VOUT], f32, kind="ExternalInput").ap()
    BOUT = nc.dram_tensor("bout", [1, 16], f32, kind="ExternalInput").ap()
    IDENT = nc.dram_tensor("ident", [128, 128], f32, kind="ExternalInput").ap()
    IOTA9 = nc.dram_tensor("iota9", [128, 32], f32, kind="ExternalInput").ap()
    ONES = nc.dram_tensor("ones", [1, 128], f32, kind="ExternalInput").ap()
    OH0 = nc.dram_tensor("oh0", [32, 128], f32, kind="ExternalInput").ap()
    PREDS = nc.dram_tensor("preds", [B_LOC, T * VOUT], f32, kind="ExternalOutput").ap()

    NG = (S + 3) // 4  # onehot transpose groups (4 steps per group, 32 rows each)

    with tile.TileContext(nc) as tc:
        _emit(nc, tc, S, T, NG, TOK, WH, ZX, ZED, WOUT, BOUT, IDENT, IOTA9, ONES, OH0, PREDS)
    nc.compile()
    return nc


def _emit(nc, tc, S, T, NG, TOK, WH, ZX, ZED, WOUT, BOUT, IDENT, IOTA9, ONES, OH0, PREDS):
    from contextlib import ExitStack

    ctx = ExitStack()
    const = ctx.enter_context(tc.tile_pool(name="const", bufs=1))

    # ---------------- phase 0: constants into SBUF (with f32 -> f32r rounding copies) ----
    tok_sb = const.tile([B_LOC, S], i32, name="tok_sb")
    nc.sync.dma_start(tok_sb[:], TOK[:])
    iota9_sb = const.tile([128, 32], f32, name="iota9_sb")
    nc.sync.dma_start(iota9_sb[:], IOTA9[:])

    wh_sb = {}
    zxr = {}
    with tc.tile_pool(name="stage", bufs=1) as stage:

        def load_r(dst, src, pslc=slice(0, 128), name="st"):
            st = stage.tile([128, dst.shape[-1]], f32, name=name, tag="st")
            nc.sync.dma_start(st[pslc, :], src)
            nc.vector.tensor_copy(dst[pslc, :], st[pslc, :])

        for d in ("f", "b", "d"):
            Wv = WH[d].rearrange("(k p) n -> k p n", p=128)
            tiles = []
            for kc in range(4):
                wt = const.tile([128, G], rdt, name=f"wh{d}{kc}")
                load_r(wt, Wv[kc], name=f"stw{d}{kc}")
                tiles.append(wt)
            wh_sb[d] = tiles

        for d in ("f", "b"):
            zt = const.tile([128, G], bf16, name=f"zxr{d}")
            st = stage.tile([128, G], f32, name=f"stzx{d}", tag="st")
            for j in range(4):
                nc.sync.dma_start(st[32 * j : 32 * j + VIN, :], ZX[d][:])
            for j in range(4):
                nc.vector.tensor_copy(
                    zt[32 * j : 32 * j + VIN, :], st[32 * j : 32 * j + VIN, :]
                )
            zxr[d] = zt

        zed_sb = const.tile([VOUT, G], bf16, name="zed_sb")
        load_r(zed_sb, ZED[:], pslc=slice(0, VOUT), name="stzed")

        wout_sb = const.tile([128, 64], rdt, name="wout_sb")
        stw = stage.tile([128, 64], f32, name="stwout", tag="st")
        nc.vector.memset(stw[:], 0.0)
        Wov = WOUT.rearrange("(k p) v -> k p v", p=128)
        for kc in range(4):
            nc.sync.dma_start(stw[:, kc * 16 : kc * 16 + VOUT], Wov[kc])
        nc.vector.tensor_copy(wout_sb[:], stw[:])

        bout_sb = const.tile([1, 16], rdt, name="bout_sb")
        load_r(bout_sb, BOUT[:], pslc=slice(0, 1), name="stbout")
        ones_sb = const.tile([1, 128], rdt, name="ones_sb")
        load_r(ones_sb, ONES[:], pslc=slice(0, 1), name="stones")
        oh0_sb = const.tile([32, 128], bf16, name="oh0_sb")
        load_r(oh0_sb, OH0[:], pslc=slice(0, 32), name="stoh0")
        ident_sb = const.tile([128, 128], bf16, name="ident_sb")
        load_r(ident_sb, IDENT[:], name="stident")
        identr_sb = const.tile([128, 128], f32r, name="identr_sb")
        load_r(identr_sb, IDENT[:], name="stidentr")

    # ---------------- phase 1: encoder one-hot tables -----------------------------------
    # oht[g][32*slot + v, b] = 1 if tokens[b, 4*g + slot] == v  (v < 16)
    ohts = const.tile([128, NG * 128], bf16, name="ohts")
    with tc.tile_pool(name="bpool", bufs=1) as bpool, tc.tile_pool(
        name="bpsum", bufs=2, space="PSUM"
    ) as bpsum:
        oh_all = bpool.tile([128, NG * 128], bf16, name="oh_all")
        ohv = oh_all[:].rearrange("p (t v) -> p t v", v=32)
        assert S % 4 == 0
        for v in range(32):
            nc.vector.tensor_scalar(ohv[:, 0:S, v], tok_sb[:], v, None, ALU.is_equal)
        for g in range(NG):
            trp = bpsum.tile([128, 128], bf16, name=f"trp{g % 2}", tag="trp")
            nc.tensor.transpose(trp[:], oh_all[:, g * 128 : (g + 1) * 128], ident_sb[:])
            nc.vector.tensor_copy(ohts[:, g * 128 : (g + 1) * 128], trp[:])

    # ---------------- phase 2+3: recurrent steps ----------------------------------------
    pz = ctx.enter_context(tc.tile_pool(name="pz", bufs=8, space="PSUM"))
    pg = ctx.enter_context(tc.tile_pool(name="pg", bufs=1))
    pc = ctx.enter_context(tc.tile_pool(name="pc", bufs=2))
    ph = ctx.enter_context(tc.tile_pool(name="ph", bufs=2))

    state = {}

    def step_mm(d, t, zx_mm, c_prev, hT_prev, wh_tiles, zpre=None):
        """Matmuls + gates + cell update for one step; returns (c_new, h2)."""
        dtag = "f" if d == "d" else d
        first = hT_prev is None
        # gate order in emission: i, g~, f, o  (tanh(c) comes off the critical path)
        GCOL = [0, 2, 1, 3]  # nt -> gate column block (i, g, f, o)
        zb = zpre
        zx_first = True
        if zb is None:
            zb = [
                pz.tile([128, 512], f32, tag="z", name=f"z{d}{t}_{nt}")
                for nt in range(4)
            ]
        for nt in range(4):
            if first:
                break
            gc = GCOL[nt]
            cols = slice(gc * 512, (gc + 1) * 512)
            for kc in range(4):
                nc.tensor.matmul(
                    zb[nt][:],
                    hT_prev[:, kc * 128 : (kc + 1) * 128],
                    wh_tiles[kc][:, cols],
                    start=(not zx_first and kc == 0),
                    stop=(zx_first and kc == 3),
                )
        if not zx_first:
            for nt in range(4):
                gc = GCOL[nt]
                zx_mm(zb[nt][:], slice(gc * 512, (gc + 1) * 512), first, first)
        gi = pg.tile([128, 512], f32, tag=f"gi{dtag}", name=f"gi{d}{t}")
        nc.scalar.activation(gi[:], zb[0][:], AF.Sigmoid)
        gg = pg.tile([128, 512], f32, tag=f"gg{dtag}", name=f"gg{d}{t}")
        nc.scalar.activation(gg[:], zb[1][:], AF.Tanh)
        gf = pg.tile([128, 512], f32, tag=f"gf{dtag}", name=f"gf{d}{t}")
        nc.scalar.activation(gf[:], zb[2][:], AF.Sigmoid)
        go = pg.tile([128, 512], f32, tag=f"go{dtag}", name=f"go{d}{t}")
        if d == "d":
            nc.scalar.activation(go[:, 0:256], zb[3][:, 0:256], AF.Sigmoid)
            nc.scalar.activation(go[:, 256:512], zb[3][:, 256:512], AF.Sigmoid)
        else:
            nc.scalar.activation(go[:], zb[3][:], AF.Sigmoid)

        c_new = pc.tile([128, 512], f32, tag=f"c{dtag}", name=f"c{d}{t}")
        if first:
            nc.vector.tensor_tensor(c_new[:], gi[:], gg[:], ALU.mult)
        else:
            t1 = pg.tile([128, 512], f32, tag=f"t1{dtag}", name=f"t1{d}{t}")
            nc.vector.tensor_tensor(t1[:], gi[:], gg[:], ALU.mult)
            nc.vector.tensor_tensor(c_new[:], gf[:], c_prev[:], ALU.mult)
            nc.vector.tensor_tensor(c_new[:], c_new[:], t1[:], ALU.add)
        tc_t = pg.tile([128, 512], f32, tag=f"tc{dtag}", name=f"tc{d}{t}")
        h2 = pg.tile([128, 512], rdt, tag=f"h2{dtag}", name=f"h2{d}{t}")
        if d == "d":
            # chunked tail: halves pipeline through ACT -> DVE -> PE transposes
            for h in range(2):
                cols = slice(h * 256, (h + 1) * 256)
                nc.scalar.activation(tc_t[:, cols], c_new[:, cols], AF.Tanh)
                nc.vector.tensor_tensor(h2[:, cols], go[:, cols], tc_t[:, cols], ALU.mult)
        else:
            nc.scalar.activation(tc_t[:], c_new[:], AF.Tanh)
            nc.vector.tensor_tensor(h2[:], go[:], tc_t[:], ALU.mult)
        return c_new, h2

    def step_tr(d, t, h2):
        """Transpose h2 -> hT (stationary layout for the next step)."""
        dtag = "f" if d == "d" else d
        trP = pz.tile([128, 512], rdt, tag="z", name=f"tr{d}{t}")
        for kc in range(4):
            nc.tensor.transpose(
                trP[:, kc * 128 : (kc + 1) * 128], h2[:, kc * 128 : (kc + 1) * 128], ident_sb[:] if RECUR_BF16 else identr_sb[:]
            )
        hT_new = ph.tile([128, 512], rdt, tag=f"hT{dtag}", name=f"hT{d}{t}")
        if d == "d":
            nc.vector.tensor_copy(hT_new[:, 0:256], trP[:, 0:256])
            nc.vector.tensor_copy(hT_new[:, 256:512], trP[:, 256:512])
        else:
            nc.vector.tensor_copy(hT_new[:], trP[:])
        return hT_new

    # encoder: forward + backward interleaved; direction d's transposes are
    # emitted after the other direction's matmul burst so the PE never waits.
    def enc_zx(d, tok_step):
        g, slot = tok_step // 4, tok_step % 4
        rows = slice(32 * slot, 32 * slot + VIN)

        def zx_mm(pdst, cols, is_first, lead, rows=rows, g=g, d=d, slot=slot):
            nc.tensor.matmul(
                pdst,
                ohts[rows, g * 128 : (g + 1) * 128],
                zxr[d][rows, cols],
                start=lead,
                stop=is_first,
                tile_position=(32 * slot, 0),
            )

        return zx_mm

    GCOL = [0, 2, 1, 3]

    def emit_oh(d, t, zx_mm, first):
        zb = []
        for nt in range(4):
            gc = GCOL[nt]
            zt = pz.tile([128, 512], f32, tag="z", name=f"z{d}{t}_{nt}")
            zx_mm(zt[:], slice(gc * 512, (gc + 1) * 512), first, True)
            zb.append(zt)
        return zb

    cs = {"f": None, "b": None}
    hTs = {"f": None, "b": None}
    h2s = {"f": None, "b": None}
    for t in range(S):
        zf = emit_oh("f", t, enc_zx("f", t), t == 0)
        zbk = emit_oh("b", t, enc_zx("b", S - 1 - t), t == 0)
        cs["f"], h2f = step_mm("f", t, None, cs["f"], hTs["f"], wh_sb["f"], zpre=zf)
        if t > 0:
            hTs["b"] = step_tr("b", t - 1, h2s["b"])
        cs["b"], h2s["b"] = step_mm("b", t, None, cs["b"], hTs["b"], wh_sb["b"], zpre=zbk)
        hTs["f"] = step_tr("f", t, h2f)
    hTs["b"] = step_tr("b", S - 1, h2s["b"])
    state = {"f": (cs["f"], hTs["f"]), "b": (cs["b"], hTs["b"])}

    # decoder init: sum of final fwd/bwd states
    c_d = pc.tile([128, 512], f32, tag="cf", name="cd_init")
    nc.vector.tensor_tensor(c_d[:], state["f"][0][:], state["b"][0][:], ALU.add)
    hT_d = ph.tile([128, 512], rdt, tag="hTf", name="hTd_init")
    nc.vector.tensor_tensor(hT_d[:], state["f"][1][:], state["b"][1][:], ALU.add)

    logits_all = const.tile([B_LOC, T * VOUT], f32, name="logits_all")
    psm = ctx.enter_context(tc.tile_pool(name="psm", bufs=2))

    ohT = oh0_sb
    for t in range(T):

        def zx_mm(pdst, cols, is_first, lead, ohT=ohT):
            nc.tensor.matmul(
                pdst,
                ohT[0:VOUT, :],
                zed_sb[0:VOUT, cols],
                start=lead,
                stop=True,
            )

        c_d, h2d = step_mm("d", t, zx_mm, c_d, hT_d, wh_sb["d"])
        hT_d = step_tr("d", t, h2d)

        lgP = pz.tile([128, 16], f32, tag="z", name=f"lg{t}")
        nc.tensor.matmul(lgP[:], ones_sb[0:1, :], bout_sb[0:1, :], start=True, stop=False)
        for kc in range(4):
            nc.tensor.matmul(
                lgP[:],
                hT_d[:, kc * 128 : (kc + 1) * 128],
                wout_sb[:, kc * 16 : (kc + 1) * 16],
                start=False,
                stop=(kc == 3),
            )
        lg_sb = logits_all[:, t * VOUT : (t + 1) * VOUT]
        nc.vector.tensor_copy(lg_sb, lgP[:, 0:VOUT])

        if t < T - 1:
            lmax = psm.tile([128, 8], f32, tag="lmax", name=f"lmax{t}")
            nc.vector.max(lmax[:], lg_sb)
            yidx = psm.tile([128, 8], u32, tag="yidx", name=f"yidx{t}")
            nc.vector.max_index(yidx[:], lmax[:], lg_sb)
            yf = psm.tile([128, 1], f32, tag="yf", name=f"yf{t}")
            nc.vector.tensor_copy(yf[:], yidx[:, 0:1])
            ohp = psm.tile([128, 32], f32, tag="ohp", name=f"ohp{t}")
            nc.vector.tensor_scalar(
                ohp[:], iota9_sb[:], yf[:, 0:1], None, ALU.is_equal
            )
            ohTf = psm.tile([32, 128], f32, tag="ohTf", name=f"ohTf{t}")
            for i in range(4):
                nc.vector.transpose(
                    ohTf[0:32, i * 32 : (i + 1) * 32], ohp[i * 32 : (i + 1) * 32, 0:32]
                )
            ohT = psm.tile([32, 128], bf16, tag="ohT", name=f"ohT{t}")
            nc.vector.tensor_copy(ohT[:], ohTf[:])

    # ---------------- phase 4: batched softmax over all logits --------------------------
    preds_sb = const.tile([B_LOC, T * VOUT], f32, name="preds_sb")
    exps = const.tile([B_LOC, T * VOUT], f32, name="exps")
    lmax48 = const.tile([128, T], f32, name="lmax48")
    lview = logits_all[:].rearrange("p (t v) -> p t v", v=VOUT)
    nc.vector.tensor_reduce(lmax48[:], lview, AX.X, ALU.max)
    for t in range(T):
        nc.vector.tensor_scalar(
            exps[:, t * VOUT : (t + 1) * VOUT],
            logits_all[:, t * VOUT : (t + 1) * VOUT],
            lmax48[:, t : t + 1],
            None,
            ALU.subtract,
        )
    nc.scalar.activation(exps[:], exps[:], AF.Exp)
    sums48 = const.tile([128, T], f32, name="sums48")
    eview = exps[:].rearrange("p (t v) -> p t v", v=VOUT)
    nc.vector.tensor_reduce(sums48[:], eview, AX.X, ALU.add)
    rec48 = const.tile([128, T], f32, name="rec48")
    nc.vector.reciprocal(rec48[:], sums48[:])
    for t in range(T):
        nc.vector.tensor_scalar(
            preds_sb[:, t * VOUT : (t + 1) * VOUT],
            exps[:, t * VOUT : (t + 1) * VOUT],
            rec48[:, t : t + 1],
            None,
            ALU.mult,
        )
    nc.sync.dma_start(PREDS[:], preds_sb[:])
    ctx.close()


_PROGRAM_CACHE = {}


def _get_program(S=128, T=48):
    key = (S, T)
    if key not in _PROGRAM_CACHE:
        _PROGRAM_CACHE[key] = build_program(S, T)
    return _PROGRAM_CACHE[key]


def make_in_maps(tokens, Wh_f, Wh_b, Wh_d, zx_f, zx_b, ze_d, W_out, b_out):
    B = tokens.shape[0]
    assert B % N_CORES == 0
    bl = B // N_CORES
    ident = np.eye(128, dtype=np.float32)
    iota9 = np.tile(np.arange(32, dtype=np.float32), (128, 1))
    ones = np.ones((1, 128), dtype=np.float32)
    oh0 = np.zeros((32, 128), dtype=np.float32)
    oh0[0, :] = 1.0
    common = dict(
        whf=np.ascontiguousarray(Wh_f, np.float32),
        whb=np.ascontiguousarray(Wh_b, np.float32),
        whd=np.ascontiguousarray(Wh_d, np.float32),
        zxf=zx_f, zxb=zx_b, zed=ze_d,
        wout=np.ascontiguousarray(W_out, np.float32),
        bout=np.pad(np.asarray(b_out, np.float32).reshape(1, VOUT), ((0, 0), (0, 16 - VOUT))),
        ident=ident, iota9=iota9, ones=ones, oh0=oh0,
    )
    return [
        {**common, "tokens": np.ascontiguousarray(tokens[c * bl : (c + 1) * bl], np.int32)}
        for c in range(N_CORES)
    ]


def fold_tables(emb_in, Wi_f, b_f, Wi_b, b_b, emb_out, Wi_d, b_d):
    f8 = lambda x: np.asarray(x, np.float64)
    zx_f = (f8(emb_in) @ f8(Wi_f) + f8(b_f)).astype(np.float32)
    zx_b = (f8(emb_in) @ f8(Wi_b) + f8(b_b)).astype(np.float32)
    ze_d = (f8(emb_out) @ f8(Wi_d) + f8(b_d)).astype(np.float32)
    return zx_f, zx_b, ze_d


def kernel(tokens, emb_in, Wi_f, Wh_f, b_f, Wi_b, Wh_b, b_b,
           emb_out, Wi_d, Wh_d, b_d, W_out, b_out, max_length):
    T = int(max_length)
    tokens = np.asarray(tokens, np.int32)
    B, S = tokens.shape
    zx_f, zx_b, ze_d = fold_tables(emb_in, Wi_f, b_f, Wi_b, b_b, emb_out, Wi_d, b_d)
    nc = _get_program(S, T)
    in_maps = make_in_maps(tokens, Wh_f, Wh_b, Wh_d, zx_f, zx_b, ze_d, W_out, b_out)
    res = run_bass_kernel_spmd(nc, in_maps, list(range(N_CORES)))
    bl = B // N_CORES
    preds = np.concatenate(
        [res.results[c]["preds"].reshape(bl, T, VOUT) for c in range(N_CORES)], axis=0
    )
    return np.ascontiguousarray(preds, np.float32)
